# revision 1
# baseline (speedup 1.0000x reference)
"""DeepVCP forward kernel for Trainium2, SPMD over 8 NeuronCores.

Pipeline (all compute on-device; host only marshals/reshapes):
  - Feature-extraction MLPs (PE matmuls) for src/tgt clouds.
  - Saliency scores + top-64 keypoints (DVE max8/max_index/match_replace).
  - Ball-query grouping via masked index sort (max8 rounds).
  - src-DFE mini-PointNet with one-hot gather matmuls; max-pool -> src_emb.
  - Voxel grid candidates (C=1331 per keypoint), exact 1-NN against the
    target cloud via an augmented matmul s = 2*c.t - |t|^2 (argmax_n s =
    argmin_n d), fused DVE copy+row-max, gpsimd index extraction, and an
    indirect-DMA gather of per-target DFE-linear vectors q_n.
  - DFE layer 2 + sim on PE, softmax + weighted candidate sum (CPG).

Sharding: the 128 (b,k) pairs are split 16 per core via a per-core
selection-matrix input; every core runs the identical program and the
tiny front-end redundantly. No collectives.
"""

import numpy as np

import concourse.bass as bass
import concourse.bacc as bacc_mod
import concourse.mybir as mybir
import concourse.tile as tile
from concourse.bass import IndirectOffsetOnAxis

F32 = mybir.dt.float32
U32 = mybir.dt.uint32
AL = mybir.AluOpType
AF = mybir.ActivationFunctionType
AX = mybir.AxisListType

B, N, K, NS = 2, 1024, 64, 32
RADIUS2 = 1.0
R_VOX, S_VOX = 2.0, 0.4
NV = 11
C = NV ** 3            # 1331
F = 32
NCHUNK = 11            # candidate chunks of 128
CPAD = NCHUNK * 128    # 1408
NCORES = 8
BKPC = (B * K) // NCORES  # 16 (b,k) pairs per core
NEG = -1.0e30


def _grid_T():
    off = np.linspace(-R_VOX, R_VOX, NV, dtype=np.float32)
    gx, gy, gz = np.meshgrid(off, off, off, indexing="ij")
    g = np.stack([gx, gy, gz], -1).reshape(-1, 3)  # (1331, 3)
    gp = np.zeros((CPAD, 3), np.float32)
    gp[:C] = g
    return np.ascontiguousarray(gp.T)  # (3, 1408)



def _chained_barrier(tc, nc, dep_insts, group=1):
    """Join many DMA completions without exceeding per-instruction sync-wait
    slots: a chain of sync-engine nops, each waiting on <= `group` DMAs
    (same-engine program order carries the rest transitively). The final nop
    becomes the Tile barrier target so every later instruction orders after
    the whole set via a single semaphore."""
    from concourse.tile import add_dep_helper
    curr_bb = nc.cur_bb
    assert curr_bb is not None
    last = None
    for i in range(0, len(dep_insts), group):
        nop = nc.sync.nop()
        for d in dep_insts[i:i + group]:
            add_dep_helper(nop.ins, d.ins, sync=True,
                           reason="chained barrier: dma join")
        last = nop
    if last is None:
        last = nc.sync.nop()
    tc.barrier_instruction_and_bb = (last.ins, curr_bb)
    if (tc.no_sync_barrier_and_bb is not None
            and tc.no_sync_barrier_and_bb[1] == curr_bb):
        tc.no_sync_barrier_and_bb = None


def build_program():
    nc = bacc_mod.Bacc(None, target_bir_lowering=False, num_swdge_queues=4)

    # ---- external inputs ------------------------------------------------
    def inp(name, shape):
        return nc.dram_tensor(name, list(shape), F32, kind="ExternalInput")

    src_h = inp("src_pts", (B, 6, N))
    tgt_h = inp("tgt_pts", (B, 6, N))
    rt_h = inp("RT", (3, 3))            # R_init[0].T (host-marshalled)
    w1_h = inp("W1", (6, F)); b1_h = inp("b1", (F,))
    w2_h = inp("W2", (F, F)); b2_h = inp("b2", (F,))
    ww1_h = inp("Ww1", (F, 16)); bw1_h = inp("bw1", (16,))
    ww2_h = inp("Ww2", (16, 1)); bw2_h = inp("bw2", (1,))
    wd1_h = inp("Wd1", (3 + F, F)); bd1_h = inp("bd1", (F,))
    wd2_h = inp("Wd2", (F, F)); bd2_h = inp("bd2", (F,))
    sel_h = inp("sel", (B * K, BKPC))    # per-core one-hot columns
    nbs_h = inp("nbasef", (128, 1))      # per-core: b*1024 (f32)
    fsel_h = inp("fsel", (4, 1))         # per-core: batch flag (f32)

    out_kxyz_h = nc.dram_tensor("out_kxyz", [BKPC, 3], F32, kind="ExternalOutput")
    out_vcp_h = nc.dram_tensor("out_vcp", [BKPC, 3], F32, kind="ExternalOutput")
    dbg = {}
    if DEBUG_TAPS:
        for nm, shp in (("dbg_score", [2, N]), ("dbg_kidx", [2, K]),
                        ("dbg_kxyzT6", [128, 6]), ("dbg_qT", [128, F]),
                        ("dbg_augTm", [4, N]), ("dbg_mkT", [3, BKPC]),
                        ("dbg_semb", [F, 128]), ("dbg_qg", [128, NCHUNK * F]),
                        ("dbg_idxf", [128, NCHUNK]), ("dbg_mbuf", [128, NCHUNK]),
                        ("dbg_simtab", [BKPC, CPAD])):
            dbg[nm] = nc.dram_tensor(nm, shp, F32, kind="ExternalOutput")

    # ---- internal DRAM tables ------------------------------------------
    qT_dram = nc.dram_tensor("qT_dram", [B * N, F], F32)
    srcT_dram = nc.dram_tensor("srcT_dram", [B * N, 6], F32)

    # ---- inline constants ----------------------------------------------
    gridT = _grid_T()                                    # (3, 1408)
    c_grid = nc.inline_tensor(gridT, "c_grid")
    g16 = [nc.inline_tensor(np.ascontiguousarray(np.tile(gridT[a], (BKPC, 1))),
                            f"c_g16_{a}") for a in range(3)]
    c_idm = nc.inline_tensor(np.eye(128, dtype=np.float32), "c_idm")
    c_ones31 = nc.inline_tensor(np.ones((3, 1), np.float32), "c_ones31")
    c_onesrow = nc.inline_tensor(np.ones((1, 512), np.float32), "c_onesrow")
    c_negones = nc.inline_tensor(np.full((1, 512), -1.0, np.float32), "c_negones")
    c_iota64r = nc.inline_tensor(np.arange(64, dtype=np.float32)[None, :].copy(),
                                 "c_iota64r")
    c_iotac = nc.inline_tensor(
        (np.arange(128, dtype=np.float32)[:, None]
         + 128.0 * np.arange(8, dtype=np.float32)[None, :]).copy(), "c_iotac")
    c_i64inv = nc.inline_tensor(
        np.tile(64.0 - np.arange(64, dtype=np.float32)[None, :], (64, 1)).copy(),
        "c_i64inv")
    c_iotab = nc.inline_tensor(
        np.tile(np.arange(N, dtype=np.float32)[None, :], (128, 1)).copy(),
        "c_iotab")
    c_nb01 = nc.inline_tensor(np.array([[0.0], [1024.0]], np.float32), "c_nb01")
    c_nboff = nc.inline_tensor(
        np.repeat(np.array([0.0, 1024.0], np.float32), 64)[:, None].copy(),
        "c_nboff")
    c_ones1408 = nc.inline_tensor(np.ones((1, CPAD), np.float32), "c_ones1408")
    kind = np.zeros((64, 2048), np.float32)
    for j in range(64):
        kind[j, j * NS:(j + 1) * NS] = 1.0
    c_kind = nc.inline_tensor(kind, "c_kind")

    load_dmas = []

    def ldma(out_ap, in_ap):
        load_dmas.append(nc.sync.dma_start(out_ap, in_ap))

    with tile.TileContext(nc) as tc:
        with tc.tile_pool(name="keep", bufs=1) as keep:
            # ==== persistent tiles (live across the whole kernel) ====
            idm = keep.tile([128, 128], F32, tag="idm")
            grids = keep.tile([3, CPAD], F32, tag="grids")
            g16t = [keep.tile([BKPC, CPAD], F32, tag=f"g16_{a}", name=f"g16_{a}")
                    for a in range(3)]
            iotab = keep.tile([128, N], F32, tag="iotab")
            nbs = keep.tile([128, 1], F32, tag="nbs")
            wd1aug = keep.tile([4, F], F32, tag="wd1aug")      # [Wd1xyz; bd1]
            wd2aug = keep.tile([F + 1, F], F32, tag="wd2aug")  # [Wd2; bd2]
            augTm = keep.tile([4, N], F32, tag="augTm")
            mkT = keep.tile([3, BKPC], F32, tag="mkT")     # rotated keypts^T
            mk16 = keep.tile([BKPC, 3], F32, tag="mk16")   # rotated keypts
            mseb = keep.tile([F, BKPC], F32, tag="mseb")   # my src_emb cols
            simtab = keep.tile([BKPC, CPAD], F32, tag="simtab")
            onesrow = keep.tile([1, 512], F32, tag="onesrow")
            bd2r = keep.tile([1, F], F32, tag="bd2r")
            # manual ping-pong tiles with a preset ones-row
            augCpp = [keep.tile([4, CPAD], F32, tag=f"augC{i}", name=f"augC{i}") for i in range(2)]

            for t, h in ((idm, c_idm), (grids, c_grid), (iotab, c_iotab),
                         (nbs, nbs_h), (onesrow, c_onesrow)):
                ldma(t[:], h[:])
            for a in range(3):
                ldma(g16t[a][:], g16[a][:])
            ldma(wd1aug[0:3, :], wd1_h[0:3, :])
            ldma(wd1aug[3:4, :], bd1_h[:])
            ldma(wd2aug[0:F, :], wd2_h[:])
            ldma(wd2aug[F:F + 1, :], bd2_h[:])
            ldma(bd2r[:], bd2_h[:])
            for t in augCpp:
                ldma(t[3:4, :], c_ones1408[:])

            # =========== front-end (tiny, replicated on all cores) ===========
            with tc.tile_pool(name="fe", bufs=1) as fe, \
                 tc.tile_pool(name="feB", bufs=3, space="PSUM") as feB, \
                 tc.tile_pool(name="feW", bufs=1, space="PSUM") as feW:

                ones31 = fe.tile([3, 1], F32, tag="ones31")
                sels = fe.tile([128, BKPC], F32, tag="sels")
                fsels = fe.tile([4, 1], F32, tag="fsels")
                wd1s = fe.tile([3 + F, F], F32, tag="wd1s")
                wd1r = fe.tile([3 + F, F], F32, tag="wd1r")
                augT = [fe.tile([4, N], F32, tag=f"augT{b}", name=f"augT{b}") for b in range(B)]
                w1s = fe.tile([6, F], F32, tag="w1s")
                w2s = fe.tile([F, F], F32, tag="w2s")
                ww1s = fe.tile([F, 16], F32, tag="ww1s")
                ww2s = fe.tile([16, 1], F32, tag="ww2s")
                b1s = fe.tile([F, 1], F32, tag="b1s")
                b2s = fe.tile([F, 1], F32, tag="b2s")
                bw1s = fe.tile([16, 1], F32, tag="bw1s")
                bw2s = fe.tile([1, 1], F32, tag="bw2s")
                rts = fe.tile([3, 3], F32, tag="rts")
                negrow = fe.tile([1, 512], F32, tag="negrow")
                iota64r = fe.tile([1, 64], F32, tag="iota64r")
                iotac = fe.tile([128, 8], F32, tag="iotac")
                i64inv = fe.tile([64, 64], F32, tag="i64inv")
                nb01 = fe.tile([2, 1], F32, tag="nb01")
                kindt = fe.tile([64, 2048], F32, tag="kindt")
                wd1aS = fe.tile([4, F], F32, tag="wd1aS")  # [-Wd1xyz; bd1]

                for t, h in ((ones31, c_ones31),
                             (sels, sel_h), (fsels, fsel_h), (wd1s, wd1_h),
                             (w1s, w1_h), (w2s, w2_h), (ww1s, ww1_h),
                             (ww2s, ww2_h), (rts, rt_h), (negrow, c_negones),
                             (iota64r, c_iota64r), (iotac, c_iotac),
                             (i64inv, c_i64inv), (nb01, c_nb01),
                             (kindt, c_kind)):
                    ldma(t[:], h[:])
                ldma(b1s[:], b1_h[:])
                ldma(b2s[:], b2_h[:])
                ldma(bw1s[:], bw1_h[:])
                ldma(bw2s[:], bw2_h[:])
                ldma(wd1aS[3:4, :], bd1_h[:])
                ldma(wd1r[0:F, :], wd1_h[3:3 + F, :])
                ldma(wd1r[F:F + 3, :], wd1_h[0:3, :])
                wd1aS_pending = True

                srcp = [fe.tile([6, N], F32, tag=f"srcp{b}", name=f"srcp{b}") for b in range(B)]
                tgtp = [fe.tile([6, N], F32, tag=f"tgtp{b}", name=f"tgtp{b}") for b in range(B)]
                for b in range(B):
                    ldma(srcp[b][:], src_h[b, :, :])
                    ldma(tgtp[b][:], tgt_h[b, :, :])

                featS = [fe.tile([F, N], F32, tag=f"featS{b}", name=f"featS{b}") for b in range(B)]
                bigT = [fe.tile([3 + F, N], F32, tag=f"bigT{b}", name=f"bigT{b}")
                        for b in range(B)]
                scoretab = fe.tile([2, N], F32, tag="scoretab")
                bd1row = fe.tile([1, F], F32, tag="bd1row")
                ldma(bd1row[:], bd1_h[:])

                # collapse the input-load burst into one sync point so no
                # matmul needs more than one fresh semaphore wait
                tc.strict_bb_all_engine_barrier()
                nc.vector.tensor_scalar(wd1aS[0:3, :], wd1s[0:3, :], -1.0,
                                        None, AL.mult)

                def fe_mlp(ptsb, out_ap):
                    """relu(relu(pts^T W1 + b1) W2 + b2)^T -> out_ap [F, N]"""
                    h_ps = feW.tile([F, N], F32, tag="feW")
                    for s in range(2):
                        nc.tensor.matmul(h_ps[:, s * 512:(s + 1) * 512], w1s[:],
                                         ptsb[:, s * 512:(s + 1) * 512])
                    h_sb = fe.tile([F, N], F32, tag="fe_hsb")
                    nc.scalar.activation(h_sb[:], h_ps[:], AF.Relu, bias=b1s[:])
                    f_ps = feW.tile([F, N], F32, tag="feW")
                    for s in range(2):
                        nc.tensor.matmul(f_ps[:, s * 512:(s + 1) * 512], w2s[:],
                                         h_sb[:, s * 512:(s + 1) * 512])
                    nc.scalar.activation(out_ap, f_ps[:], AF.Relu, bias=b2s[:])

                for b in range(B):
                    fe_mlp(srcp[b], featS[b][:])
                    fe_mlp(tgtp[b], bigT[b][0:F, :])

                # ---- tgt: augT rows (2t, -|t|^2), bigT rows (-t), qT ----
                tbl_dmas = []
                for b in range(B):
                    nc.scalar.activation(augT[b][0:3, :], tgtp[b][0:3, :],
                                         AF.Copy, scale=2.0)
                    nc.scalar.activation(bigT[b][F:F + 3, :], tgtp[b][0:3, :],
                                         AF.Copy, scale=-1.0)
                    sqt = fe.tile([3, N], F32, tag="sqt")
                    nc.vector.tensor_tensor(sqt[:], tgtp[b][0:3, :],
                                            tgtp[b][0:3, :], AL.mult)
                    for s in range(2):
                        tn_ps = feB.tile([1, 512], F32, tag="feB")
                        nc.tensor.matmul(tn_ps[:], ones31[:],
                                         sqt[:, s * 512:(s + 1) * 512])
                        tn_sb = fe.tile([1, 512], F32, tag="tn_sb")
                        nc.scalar.activation(tn_sb[:], tn_ps[:], AF.Copy,
                                             scale=-1.0)
                        nc.sync.dma_start(
                            augT[b][3:4, s * 512:(s + 1) * 512], tn_sb[:])
                    for c8 in range(8):
                        q_ps = feB.tile([128, F], F32, tag="feB")
                        nc.tensor.matmul(q_ps[:],
                                         bigT[b][:, c8 * 128:(c8 + 1) * 128],
                                         wd1r[:])
                        q_sb = fe.tile([128, F], F32, tag="qsb")
                        nc.scalar.activation(q_sb[:], q_ps[:], AF.Copy)
                        tbl_dmas.append(nc.sync.dma_start(
                            qT_dram[b * N + c8 * 128: b * N + (c8 + 1) * 128, :],
                            q_sb[:]))

                # ---- srcT table (for keypoint xyz gather) ----
                for b in range(B):
                    for c8 in range(8):
                        tr_ps = feB.tile([128, 6], F32, tag="feB")
                        nc.tensor.transpose(
                            tr_ps[:], srcp[b][:, c8 * 128:(c8 + 1) * 128],
                            idm[0:6, 0:6])
                        tr_sb = fe.tile([128, 6], F32, tag="trsb")
                        nc.scalar.activation(tr_sb[:], tr_ps[:], AF.Copy)
                        tbl_dmas.append(nc.sync.dma_start(
                            srcT_dram[b * N + c8 * 128: b * N + (c8 + 1) * 128,
                                      :],
                            tr_sb[:]))

                # ---- saliency scores + top-64 ----
                for b in range(B):
                    h3_ps = feW.tile([16, N], F32, tag="feW")
                    for s in range(2):
                        nc.tensor.matmul(h3_ps[:, s * 512:(s + 1) * 512],
                                         ww1s[:],
                                         featS[b][:, s * 512:(s + 1) * 512])
                    h3_sb = fe.tile([16, N], F32, tag="h3sb")
                    nc.scalar.activation(h3_sb[:], h3_ps[:], AF.Relu,
                                         bias=bw1s[:])
                    for s in range(2):
                        sc_ps = feB.tile([1, 512], F32, tag="feB")
                        nc.tensor.matmul(sc_ps[:], ww2s[:],
                                         h3_sb[:, s * 512:(s + 1) * 512])
                        sc_sb = fe.tile([1, 512], F32, tag="sc_sb")
                        nc.scalar.activation(sc_sb[:], sc_ps[:], AF.Identity,
                                             bias=bw2s[:])
                        nc.sync.dma_start(
                            scoretab[b:b + 1, s * 512:(s + 1) * 512],
                            sc_sb[:])

                if DEBUG_TAPS:
                    nc.sync.dma_start(dbg["dbg_score"][:], scoretab[:])
                kidx_u = fe.tile([2, K], U32, tag="kidxu")
                m8k = fe.tile([2, 8], F32, tag="m8k")
                for r in range(8):
                    nc.vector.max(m8k[:], scoretab[:])
                    nc.vector.max_index(kidx_u[:, r * 8:(r + 1) * 8], m8k[:],
                                        scoretab[:])
                    nc.vector.match_replace(scoretab[:], m8k[:], scoretab[:],
                                            NEG)
                kidxf = fe.tile([2, K], F32, tag="kidxf")
                nc.vector.tensor_copy(kidxf[:], kidx_u[:])
                k128f = fe.tile([128, 1], F32, tag="k128f")
                nc.sync.dma_start(k128f[:], kidxf[:])
                nboff = fe.tile([128, 1], F32, tag="nboff")
                nc.sync.dma_start(nboff[:], c_nboff[:])
                kidx_off = fe.tile([128, 1], U32, tag="kidxoff")
                nc.vector.tensor_scalar(kidx_off[:], k128f[:], nboff[:], None,
                                        AL.add)
                if DEBUG_TAPS:
                    nc.sync.dma_start(dbg["dbg_kidx"][:], kidxf[:])
                # PE rhs operands need base partition 0: copy row b of kidxf
                kidxr = [fe.tile([1, K], F32, tag=f"kidxr{b}",
                                 name=f"kidxr{b}") for b in range(B)]
                for b in range(B):
                    nc.sync.dma_start(kidxr[b][:], kidxf[b:b + 1, :])

                # qT/srcT tables must be in DRAM before any gather
                tc.strict_bb_all_engine_barrier()

                # ---- gather keypoints (both batches, all 128) ----
                kxyzT6 = fe.tile([128, 6], F32, tag="kxyzT6")
                nc.gpsimd.indirect_dma_start(
                    kxyzT6[:], None, srcT_dram[:, :],
                    IndirectOffsetOnAxis(ap=kidx_off[:], axis=0))

                if DEBUG_TAPS:
                    nc.sync.dma_start(dbg["dbg_kxyzT6"][:], kxyzT6[:])
                kx_ps = feB.tile([3, 128], F32, tag="feB")
                nc.tensor.transpose(kx_ps[:], kxyzT6[:, 0:3], idm[:])
                kxyz3 = fe.tile([3, 128], F32, tag="kxyz3")
                nc.scalar.activation(kxyz3[:], kx_ps[:], AF.Copy)

                # rotated keypoints: kxyz_t^T [128,3] = kxyz^T @ R^T
                ktt_ps = feB.tile([128, 3], F32, tag="feB")
                nc.tensor.matmul(ktt_ps[:], kxyz3[:], rts[:])
                ktt_sb = fe.tile([128, 3], F32, tag="kttsb")
                nc.scalar.activation(ktt_sb[:], ktt_ps[:], AF.Copy)

                mk_ps = feB.tile([BKPC, 3], F32, tag="feB")
                nc.tensor.matmul(mk_ps[:], sels[:], ktt_sb[:])
                nc.scalar.activation(mk16[:], mk_ps[:], AF.Copy)
                mkT_ps = feB.tile([3, BKPC], F32, tag="feB")
                nc.tensor.matmul(mkT_ps[:], ktt_sb[:], sels[:])
                nc.scalar.activation(mkT[:], mkT_ps[:], AF.Copy)

                kxo_ps = feB.tile([BKPC, 3], F32, tag="feB")
                nc.tensor.matmul(kxo_ps[:], sels[:], kxyzT6[:, 0:3])
                kxo_sb = fe.tile([BKPC, 3], F32, tag="kxosb")
                nc.scalar.activation(kxo_sb[:], kxo_ps[:], AF.Copy)
                nc.sync.dma_start(out_kxyz_h[:], kxo_sb[:])

                # ---- ball query + src DFE + src_emb ----
                semb_all = fe.tile([F, 128], F32, tag="semball")
                for b in range(B):
                    kxb = kxyz3[:, b * 64:(b + 1) * 64]
                    km2 = fe.tile([3, 64], F32, tag="km2")
                    nc.vector.tensor_scalar(km2[:], kxb, -2.0, None, AL.mult)
                    kn2 = fe.tile([3, 64], F32, tag="kn2")
                    nc.vector.tensor_tensor(kn2[:], kxb, kxb, AL.mult)
                    nrm_ps = feB.tile([1, 64], F32, tag="feB")
                    nc.tensor.matmul(nrm_ps[:], ones31[:], kn2[:])
                    nrm_sb = fe.tile([1, 64], F32, tag="nrmsb")
                    nc.scalar.activation(nrm_sb[:], nrm_ps[:], AF.Copy)

                    sq_ps = feB.tile([64, 64], F32, tag="feB")
                    nc.tensor.matmul(sq_ps[:], kxb, km2[:], start=True,
                                     stop=False)
                    nc.tensor.matmul(sq_ps[:], onesrow[:, 0:64], nrm_sb[:],
                                     start=False, stop=False)
                    nc.tensor.matmul(sq_ps[:], nrm_sb[:], onesrow[:, 0:64],
                                     start=False, stop=True)

                    # v = (sqd<=1)*(64-j); top-32 desc -> neighbors ascending
                    vb = fe.tile([64, 64], F32, tag="vb")
                    nc.vector.scalar_tensor_tensor(vb[:], sq_ps[:], RADIUS2,
                                                   i64inv[:], AL.is_le,
                                                   AL.mult)
                    gidxv = fe.tile([64, NS], F32, tag="gidxv")
                    m8b = fe.tile([64, 8], F32, tag="m8b")
                    for r in range(4):
                        nc.vector.max(m8b[:], vb[:])
                        nc.vector.tensor_scalar(gidxv[:, r * 8:(r + 1) * 8],
                                                m8b[:], -1.0, 64.0, AL.mult,
                                                AL.add)
                        nc.vector.match_replace(vb[:], m8b[:], vb[:], NEG)
                    # pad (==64) -> first neighbor
                    dcol = fe.tile([64, NS], F32, tag="dcol")
                    nc.vector.tensor_scalar(dcol[:], gidxv[:], -1.0,
                                            gidxv[:, 0:1], AL.mult, AL.add)
                    m64 = fe.tile([64, NS], F32, tag="m64")
                    nc.vector.tensor_scalar(m64[:], gidxv[:], 64.0, None,
                                            AL.is_equal)
                    fixt = fe.tile([64, NS], F32, tag="fixt")
                    nc.vector.tensor_tensor(fixt[:], m64[:], dcol[:], AL.mult)
                    nc.vector.tensor_tensor(gidxv[:], gidxv[:], fixt[:],
                                            AL.add)

                    # flatten [64,32] -> [1,2048] (k-major) via sbuf-sbuf DMA
                    gflat = fe.tile([1, 64 * NS], F32, tag="gflat")
                    nc.sync.dma_start(gflat[:], gidxv[:])

                    # one-hot oh[j,(k,s)] = (gidx[k,s] == j)
                    oh = fe.tile([64, 2048], F32, tag="oh")
                    for s4 in range(4):
                        ohp = feB.tile([64, 512], F32, tag="feB")
                        nc.tensor.matmul(ohp[:], onesrow[:, 0:64],
                                         gflat[:, s4 * 512:(s4 + 1) * 512],
                                         start=True, stop=False)
                        nc.tensor.matmul(ohp[:], iota64r[:], negrow[:],
                                         start=False, stop=True)
                        nc.vector.tensor_scalar(
                            oh[:, s4 * 512:(s4 + 1) * 512],
                            ohp[:], 0.0, None, AL.is_equal)

                    # keypoint features kf [F,64] via one-hot matmuls
                    kb_ps = feW.tile([128, 64], F32, tag="kbps")
                    nc.tensor.matmul(kb_ps[:], onesrow[:, 0:128], kidxr[b][:])
                    kf_ps = feW.tile([F, 64], F32, tag="kfps")
                    for c8 in range(8):
                        ohkn = fe.tile([128, 64], F32, tag="ohkn")
                        nc.vector.tensor_scalar(ohkn[:], kb_ps[:],
                                                iotac[:, c8:c8 + 1], None,
                                                AL.is_equal)
                        ft_ps = feB.tile([128, F], F32, tag="feB")
                        nc.tensor.transpose(
                            ft_ps[:], featS[b][:, c8 * 128:(c8 + 1) * 128],
                            idm[0:F, 0:F])
                        ft_sb = fe.tile([128, F], F32, tag="ftsb")
                        nc.vector.tensor_copy(ft_sb[:], ft_ps[:])
                        nc.tensor.matmul(kf_ps[:], ft_sb[:], ohkn[:],
                                         start=(c8 == 0), stop=(c8 == 7))

                    src35 = fe.tile([3 + F, 64], F32, tag="src35")
                    nc.scalar.activation(src35[0:F, :], kf_ps[:], AF.Copy)
                    nc.scalar.activation(src35[F:F + 3, :], kxb, AF.Copy)

                    qpT_ps = feB.tile([64, F], F32, tag="feB")
                    nc.tensor.matmul(qpT_ps[:], src35[:], wd1r[:])
                    qpT_sb = fe.tile([64, F], F32, tag="qpTsb")
                    nc.vector.tensor_copy(qpT_sb[:], qpT_ps[:])

                    kxbd = fe.tile([3, 64], F32, tag="kxbd")
                    nc.vector.tensor_copy(kxbd[:], kxb)
                    psT_ps = feB.tile([64, F], F32, tag="feB")
                    nc.tensor.matmul(psT_ps[:], kxbd[:], wd1aS[0:3, :],
                                     start=True, stop=False)
                    nc.tensor.matmul(psT_ps[:], onesrow[:, 0:64], bd1row[:],
                                     start=False, stop=True)
                    psT_sb = fe.tile([64, F], F32, tag="psTsb")
                    nc.scalar.activation(psT_sb[:], psT_ps[:], AF.Copy)

                    # h1 = relu(q'g + p' bcast); h2 = relu(Wd2^T h1 + bd2)
                    h1src = fe.tile([F, 2048], F32, tag="h1src")
                    for s4 in range(4):
                        qg_ps = feB.tile([F, 512], F32, tag="feB")
                        nc.tensor.matmul(qg_ps[:], qpT_sb[:],
                                         oh[:, s4 * 512:(s4 + 1) * 512],
                                         start=True, stop=False)
                        nc.tensor.matmul(qg_ps[:], psT_sb[:],
                                         kindt[:, s4 * 512:(s4 + 1) * 512],
                                         start=False, stop=True)
                        nc.scalar.activation(
                            h1src[:, s4 * 512:(s4 + 1) * 512],
                            qg_ps[:], AF.Relu)
                    h2src = fe.tile([F, 64, NS], F32, tag="h2src")
                    for s4 in range(4):
                        h2s_ps = feB.tile([F, 512], F32, tag="feB")
                        nc.tensor.matmul(h2s_ps[:], wd2aug[0:F, :],
                                         h1src[:, s4 * 512:(s4 + 1) * 512],
                                         start=True, stop=False)
                        nc.tensor.matmul(h2s_ps[:], bd2r[:], onesrow[:],
                                         start=False, stop=True)
                        nc.scalar.activation(
                            h2src[:, s4 * 16:(s4 + 1) * 16, :].rearrange(
                                "a b c -> a (b c)"),
                            h2s_ps[:], AF.Relu)
                    nc.vector.tensor_reduce(semb_all[:, b * 64:(b + 1) * 64],
                                            h2src[:], AX.X, AL.max)

                # my src_emb columns [F,16]: transpose -> select -> transpose
                seT_ps = feB.tile([128, F], F32, tag="feB")
                nc.tensor.transpose(seT_ps[:], semb_all[:], idm[0:F, 0:F])
                seT_sb = fe.tile([128, F], F32, tag="seTsb")
                nc.scalar.activation(seT_sb[:], seT_ps[:], AF.Copy)
                mse_ps = feB.tile([BKPC, F], F32, tag="feB")
                nc.tensor.matmul(mse_ps[:], sels[:], seT_sb[:])
                mse_sb = fe.tile([BKPC, F], F32, tag="msesb")
                nc.scalar.activation(mse_sb[:], mse_ps[:], AF.Copy)
                mse2_ps = feB.tile([F, BKPC], F32, tag="feB")
                nc.tensor.transpose(mse2_ps[:], mse_sb[:], idm[0:BKPC, 0:BKPC])
                nc.scalar.activation(mseb[:], mse2_ps[:], AF.Copy)

                if DEBUG_TAPS:
                    nc.sync.dma_start(dbg["dbg_mkT"][:], mkT[:])
                    nc.sync.dma_start(dbg["dbg_semb"][:], semb_all[:])
                    qtmp = fe.tile([128, F], F32, tag="qtmp")
                    nc.sync.dma_start(qtmp[:], qT_dram[0:128, :])
                    nc.sync.dma_start(dbg["dbg_qT"][:], qtmp[:])
                # augT_mine = augT0 + fsel * (augT1 - augT0)
                d01 = fe.tile([4, N], F32, tag="d01")
                nc.vector.tensor_tensor(d01[:], augT[1][:], augT[0][:],
                                        AL.subtract)
                nc.vector.tensor_scalar(d01[:], d01[:], fsels[:], None,
                                        AL.mult)
                nc.vector.tensor_tensor(augTm[:], augT[0][:], d01[:], AL.add)

            if DEBUG_TAPS:
                nc.sync.dma_start(dbg["dbg_augTm"][:], augTm[:])
            # ================= main loop: 16 (b,k) pairs =================
            with tc.tile_pool(name="ml", bufs=2) as ml, \
                 tc.tile_pool(name="mlg", bufs=1) as mlg, \
                 tc.tile_pool(name="Sps", bufs=2, space="PSUM") as spool, \
                 tc.tile_pool(name="pacc", bufs=1, space="PSUM") as paccp, \
                 tc.tile_pool(name="ptmp", bufs=3, space="PSUM") as ptmp:
                def phase_a(j):
                    augC = augCpp[j % 2]
                    nc.scalar.activation(augC[0:3, :], grids[:], AF.Identity,
                                         bias=mkT[:, j:j + 1])

                    pacc = paccp.tile([128, NCHUNK * F], F32, tag="pacc",
                                      name="pacc")
                    mbuf = ml.tile([128, NCHUNK], F32, tag="mbuf", name="mbuf")
                    idxf = ml.tile([128, NCHUNK], F32, tag="idxf", name="idxf")
                    # chunk pairs: reduce(c), reduce(c+1), stt(c), stt(c+1)
                    # hides the mbuf write->read latency between the two
                    # dependent DVE passes of one chunk
                    for cc in range(0, NCHUNK, 2):
                        pair = [c for c in (cc, cc + 1) if c < NCHUNK]
                        S_tiles = {}
                        for c in pair:
                            lhs = augC[:, c * 128:(c + 1) * 128]
                            S_ps = spool.tile([128, N], F32, tag="S", name="S")
                            nc.tensor.matmul(S_ps[:, 0:512], lhs,
                                             augTm[:, 0:512])
                            nc.tensor.matmul(S_ps[:, 512:1024], lhs,
                                             augTm[:, 512:1024])
                            nc.tensor.matmul(pacc[:, c * F:(c + 1) * F], lhs,
                                             wd1aug[:])
                            # pass 1: per-row max of S (psum)
                            nc.vector.tensor_reduce(mbuf[:, c:c + 1], S_ps[:],
                                                    AX.X, AL.max)
                            S_tiles[c] = S_ps
                        for c in pair:
                            # pass 2: index extraction, sum((S>=m) * iota)
                            scr = mlg.tile([128, N], F32, tag="scr",
                                           name="scr")
                            nc.vector.scalar_tensor_tensor(
                                scr[:], S_tiles[c][:], mbuf[:, c:c + 1],
                                iotab[:], AL.is_ge, AL.mult,
                                accum_out=idxf[:, c:c + 1])

                    pacc_sb = ml.tile([128, NCHUNK * F], F32, tag="pacc_sb",
                                      name="pacc_sb")
                    nc.scalar.activation(pacc_sb[:], pacc[:], AF.Copy)
                    idxu = ml.tile([128, NCHUNK], U32, tag="idxu", name="idxu")
                    nc.vector.tensor_scalar(idxu[:], idxf[:], nbs[:],
                                            float(B * N - 1), AL.add, AL.min)
                    qg = ml.tile([128, NCHUNK * F], F32, tag="qg", name="qg")
                    for c in range(NCHUNK):
                        nc.gpsimd.indirect_dma_start(
                            qg[:, c * F:(c + 1) * F], None, qT_dram[:, :],
                            IndirectOffsetOnAxis(ap=idxu[:, c:c + 1], axis=0))
                    if DEBUG_TAPS and j == 0:
                        nc.sync.dma_start(dbg["dbg_qg"][:], qg[:])
                        nc.sync.dma_start(dbg["dbg_idxf"][:], idxf[:])
                        nc.sync.dma_start(dbg["dbg_mbuf"][:], mbuf[:])
                    return j, pacc_sb, qg

                def phase_b(state):
                    j, pacc_sb, qg = state
                    simrow = ml.tile([1, CPAD], F32, tag="simrow",
                                     name="simrow")
                    for c in range(NCHUNK):
                        # (p' + q)^T via two accumulated PE transposes
                        tr_ps = ptmp.tile([F, 128], F32, tag="pt", name="trps")
                        nc.tensor.matmul(tr_ps[:],
                                         pacc_sb[:, c * F:(c + 1) * F],
                                         idm[:], is_transpose=True,
                                         start=True, stop=False)
                        nc.tensor.matmul(tr_ps[:], qg[:, c * F:(c + 1) * F],
                                         idm[:], is_transpose=True,
                                         start=False, stop=True)
                        h1T = ml.tile([F, 128], F32, tag="h1T", name="h1T")
                        nc.scalar.activation(h1T[:], tr_ps[:], AF.Relu)
                        h2_ps = ptmp.tile([F, 128], F32, tag="pt", name="h2ps")
                        nc.tensor.matmul(h2_ps[:], wd2aug[0:F, :], h1T[:],
                                         start=True, stop=False)
                        nc.tensor.matmul(h2_ps[:], bd2r[:], onesrow[:, 0:128],
                                         start=False, stop=True)
                        h2sb = ml.tile([F, 128], F32, tag="h2sb", name="h2sb")
                        nc.scalar.activation(h2sb[:], h2_ps[:], AF.Relu)
                        sim_ps = ptmp.tile([1, 128], F32, tag="pt",
                                           name="sim_ps")
                        nc.tensor.matmul(sim_ps[:], mseb[:, j:j + 1], h2sb[:])
                        nc.scalar.activation(
                            simrow[:, c * 128:(c + 1) * 128],
                            sim_ps[:], AF.Copy)
                    nc.sync.dma_start(simtab[j:j + 1, :], simrow[:])

                prev = None
                for j in range(BKPC):
                    st = phase_a(j)
                    if prev is not None:
                        phase_b(prev)
                    prev = st
                phase_b(prev)

            if DEBUG_TAPS:
                nc.sync.dma_start(dbg["dbg_simtab"][:], simtab[:])
            # ================= softmax + CPG epilogue =================
            with tc.tile_pool(name="ep", bufs=1) as ep:
                nc.vector.memset(simtab[:, C:CPAD], NEG)
                rmax = ep.tile([BKPC, 1], F32, tag="rmax")
                nc.vector.tensor_reduce(rmax[:], simtab[:], AX.X, AL.max)
                nmax = ep.tile([BKPC, 1], F32, tag="nmax")
                nc.vector.tensor_scalar(nmax[:], rmax[:], -1.0, None, AL.mult)
                wtab = ep.tile([BKPC, CPAD], F32, tag="wtab")
                sums = ep.tile([BKPC, 1], F32, tag="sums")
                nc.scalar.activation(wtab[:], simtab[:], AF.Exp, bias=nmax[:],
                                     accum_out=sums[:])
                rs = ep.tile([BKPC, 1], F32, tag="rs")
                nc.vector.reciprocal(rs[:], sums[:])
                vcp_pre = ep.tile([BKPC, 3], F32, tag="vcppre")
                scr2 = ep.tile([BKPC, CPAD], F32, tag="scr2")
                for a in range(3):
                    cax = ep.tile([BKPC, CPAD], F32, tag="cax")
                    nc.vector.tensor_scalar(cax[:], g16t[a][:],
                                            mk16[:, a:a + 1], None, AL.add)
                    nc.vector.scalar_tensor_tensor(
                        scr2[:], wtab[:], 1.0, cax[:],
                        AL.mult, AL.mult, accum_out=vcp_pre[:, a:a + 1])
                vcp_sb = ep.tile([BKPC, 3], F32, tag="vcpsb")
                nc.vector.tensor_scalar(vcp_sb[:], vcp_pre[:], rs[:], None,
                                        AL.mult)
                nc.sync.dma_start(out_vcp_h[:], vcp_sb[:])

    nc.compile()
    return nc


def _in_maps(inputs):
    def f32(x):
        return np.ascontiguousarray(np.asarray(x, dtype=np.float32))

    base = {
        "src_pts": f32(inputs["src_pts"]),
        "tgt_pts": f32(inputs["tgt_pts"]),
        "RT": f32(np.asarray(inputs["R_init"])[0].T),
        "W1": f32(inputs["W1"]), "b1": f32(inputs["b1"]),
        "W2": f32(inputs["W2"]), "b2": f32(inputs["b2"]),
        "Ww1": f32(inputs["Ww1"]), "bw1": f32(inputs["bw1"]),
        "Ww2": f32(inputs["Ww2"]), "bw2": f32(inputs["bw2"]),
        "Wd1": f32(inputs["Wd1"]), "bd1": f32(inputs["bd1"]),
        "Wd2": f32(inputs["Wd2"]), "bd2": f32(inputs["bd2"]),
    }
    maps = []
    for core in range(NCORES):
        sel = np.zeros((B * K, BKPC), np.float32)
        for i in range(BKPC):
            sel[core * BKPC + i, i] = 1.0
        b = core // (NCORES // B)
        m = dict(base)
        m["sel"] = sel
        m["nbasef"] = np.full((128, 1), 1024.0 * b, np.float32)
        m["fsel"] = np.full((4, 1), float(b), np.float32)
        maps.append(m)
    return maps


DEBUG_TAPS = False

_PROG = None


def kernel(**inputs):
    global _PROG
    from concourse.bass_utils import run_bass_kernel_spmd
    if _PROG is None:
        _PROG = build_program()
    res = run_bass_kernel_spmd(_PROG, _in_maps(inputs),
                               list(range(NCORES))).results
    kxyz = np.concatenate([res[i]["out_kxyz"] for i in range(NCORES)],
                          axis=0).reshape(B, K, 3)
    vcp = np.concatenate([res[i]["out_vcp"] for i in range(NCORES)],
                         axis=0).reshape(B, K, 3)
    return kxyz, vcp

